# revision 43
# baseline (speedup 1.0000x reference)
"""Trainium2 Bass kernel for nn_MLP_Route_RL_Model (route RL model).

Reference math (per batch element b of 256):
  - state = [route_nums (48) | customers (48*24*36)]
  - customer MLP (tanh-tanh, 36->128->32) on every node of every route
  - 2-layer GRU (hidden 128) over the 24 nodes of each of the 48 routes
  - route summary mean, node-selection MLP 256->256->128->24, masked softmax

Sharding: pure data parallel over batch B=256 -> 8 cores x 32.

Layout: feature-major activations ([feature, token] in SBUF) so matmuls
contract over the partition dim without transposes.

Schedule notes (the kernel is ACT elementwise-bound; engine cost is
free-dim elements only, so fewer/bigger ACT ops win):
  - r|z gate pre-activations share one 2-bank PSUM tile [128,1024]; with
    the (all-zero) biases dropped, ONE sigmoid covers both gates.
  - n-gate input finishes inside PSUM: after the DVE computes
    t_ = r*ph, an identity matmul accumulates I @ t_ onto the x-side
    matmul in the pi bank, deleting the separate s_ = pi + t_ DVE add.
  - h' = n + z*(h-n): 3 DVE ops/chunk at fp16 2x.
  - PSUM tags: przb (2 banks x2 slots) + ph (x2) + pi (x2) = 8 banks;
    the customer MLP borrows przb/pi slots at low scheduler priority and
    runs one 4-node group ahead of the GRU.
  - All fp16 weights ship in ONE packed DMA (SBUF views), fp32 consts in
    another; cust loads are per-group; outputs merge to one DMA per
    512-token chunk. DMA issue occupies the SP sequencer 565ns each, so
    instruction count matters.
  - A non-zero-bias fallback keeps split sigmoids + bias ports.
"""

import contextlib
import sys

import numpy as np

sys.path.insert(0, "/opt/trn_rl_repo")

import concourse.bass as bass  # noqa: E402
import concourse.bacc as bacc  # noqa: E402
import concourse.mybir as mybir  # noqa: E402
import concourse.tile as tile  # noqa: E402
from concourse.bass_utils import run_bass_kernel_spmd  # noqa: E402

F32 = mybir.dt.float32
F16 = mybir.dt.float16
AF = mybir.ActivationFunctionType
OP = mybir.AluOpType

# Problem shape constants
B = 256
NCORES = 8
BLOC = B // NCORES          # 32 batch rows per core
MR = 48                     # routes per batch
MN = 24                     # nodes per route
FEAT = 36
CH = 128                    # customer hidden
CO = 32                     # customer out
GH = 128                    # GRU hidden
S = BLOC * MR               # sequences per core = 1536
NC = 512                    # token chunk (PSUM bank width in fp32)
NCH = S // NC               # chunks per core = 3
NG = MN // 4                # node groups of 4 (cust_out partition stacking)
NQ = S // 128               # 128-token groups per core = 12

# fp16 weight pack layout: (col_offset, n_cols, n_rows)
_P16 = {}
_c = 0
for _name, _ncol, _nrow in [
    ("wc1", CH, FEAT), ("wc2", CO, CH), ("ident", 128, 128),
    ("wih0", 3 * GH, 128), ("whh0", 3 * GH, GH),
    ("wih1", 3 * GH, GH), ("whh1", 3 * GH, GH),
    ("wn1a", 256, GH), ("wn1b", 256, GH),
    ("wn2a", 128, 128), ("wn2b", 128, 128), ("wn3", MN, GH),
    ("sel", S, BLOC),
]:
    _P16[_name] = (_c, _ncol, _nrow)
    _c += _ncol
COLS16 = _c
# fp32 const pack layout
_P32 = {"iota24": (0, MN, 128), "rn_pm": (MN, NQ, 128)}
COLS32 = MN + NQ

_cache = {}


def _build(reps=1, zb=True):
    """Trace + schedule the per-core Tile kernel. zb: all biases are zero."""
    nc = bacc.Bacc("TRN2", target_bir_lowering=False, debug=False)

    # ---- DRAM I/O ----------------------------------------------------------
    d_cust = nc.dram_tensor("cust_fm", [FEAT, MN * S], F16, kind="ExternalInput")
    d_wmini = nc.dram_tensor("wmini", [128, CH + CO], F16, kind="ExternalInput")
    d_w16 = nc.dram_tensor("wpk16", [128, COLS16], F16, kind="ExternalInput")
    d_c32 = nc.dram_tensor("cpk32", [128, COLS32], F32, kind="ExternalInput")
    if not zb:
        d_bc1 = nc.dram_tensor("bc1", [CH, 1], F32, kind="ExternalInput")
        d_bc2 = nc.dram_tensor("bc2s", [128, 1], F32, kind="ExternalInput")
        d_gb = {}
        for layer in (0, 1):
            for g in ("r", "z", "in", "hn"):
                d_gb[(layer, g)] = nc.dram_tensor(
                    f"b{layer}_{g}", [GH, 1], F32, kind="ExternalInput"
                )
        d_bn1 = nc.dram_tensor("bn1c", [128, 2], F32, kind="ExternalInput")
        d_bn2 = nc.dram_tensor("bn2c", [128, 1], F32, kind="ExternalInput")
        d_bn3 = nc.dram_tensor("bn3r", [1, MN], F32, kind="ExternalInput")
        d_ones = nc.dram_tensor("ones128", [1, 128], F32, kind="ExternalInput")
    # partition-major output: out_pm[p, q*MN+m] = probs for token q*128+p
    d_out = nc.dram_tensor("out_pm", [128, NQ * MN], F32, kind="ExternalOutput")

    with tile.TileContext(nc) as tc:
        with (
            tc.tile_pool(name="wpool", bufs=1) as wp,
            tc.tile_pool(name="state", bufs=1) as sp,
            tc.tile_pool(name="xin", bufs=4) as xp,
            tc.tile_pool(name="h1c", bufs=10) as h1p,
            tc.tile_pool(name="wk", bufs=12) as wk,
            tc.tile_pool(name="fin", bufs=4) as fp_,
            tc.tile_pool(name="ps2", bufs=2, space="PSUM") as ps2,
            tc.tile_pool(name="ps1", bufs=2, space="PSUM") as ps1,
        ):
            def lowprio():
                # deprioritize: scheduler runs these only in recurrence gaps
                return tc.high_priority(offset=-1_000_000)

            # ---- packed weights / constants ---------------------------------
            # customer-MLP weights ship in a small DMA ahead of the big pack
            # so phase A's first matmul isn't gated on the full 1MB transfer.
            wmini = wp.tile([128, CH + CO], F16, tag="wmini")
            nc.sync.dma_start(wmini[:], d_wmini.ap())
            wpk = wp.tile([128, COLS16], F16, tag="wpk16")
            nc.sync.dma_start(wpk[:], d_w16.ap())
            cpk = wp.tile([128, COLS32], F32, tag="cpk32")
            nc.sync.dma_start(cpk[:], d_c32.ap())

            def w16(name, r0=0, rows=None, c0=0, cols=None):
                off, ncol, nrow = _P16[name]
                rows = nrow if rows is None else rows
                cols = ncol if cols is None else cols
                return wpk[r0 : r0 + rows, off + c0 : off + c0 + cols]

            wc1 = wmini[0:FEAT, 0:CH]
            wc2 = wmini[0:CH, CH : CH + CO]
            ident = w16("ident")
            # GRU weight gate slices; L1 x-side per partition-quadrant k
            whh = {
                (0, g): w16("whh0", c0=g * GH, cols=GH) for g in range(3)
            }
            whh.update({
                (1, g): w16("whh1", c0=g * GH, cols=GH) for g in range(3)
            })
            wih1g = {g: w16("wih1", c0=g * GH, cols=GH) for g in range(3)}
            wih0g = {
                (k, g): w16("wih0", r0=32 * k, rows=CO, c0=g * GH, cols=GH)
                for k in range(4) for g in range(3)
            }
            wn1a = {m: w16("wn1a", c0=128 * m, cols=128) for m in range(2)}
            wn1b = w16("wn1b")
            wn2a = w16("wn2a")
            wn2b = w16("wn2b")
            wn3 = w16("wn3")
            selc = {c: w16("sel", c0=c * NC, cols=NC) for c in range(NCH)}
            iota24 = cpk[0:128, 0:MN]
            rnq = {q: cpk[0:128, MN + q : MN + q + 1] for q in range(NQ)}

            gb = {}
            if not zb:
                def wtile(dram, shape, dtype):
                    t = wp.tile(shape, dtype, tag=dram.name)
                    nc.sync.dma_start(t[:], dram.ap())
                    return t
                bc1 = wtile(d_bc1, [CH, 1], F32)
                bc2 = wtile(d_bc2, [128, 1], F32)
                for k, d in d_gb.items():
                    gb[k] = wtile(d, [GH, 1], F32)
                bn1 = wtile(d_bn1, [128, 2], F32)
                bn2 = wtile(d_bn2, [128, 1], F32)
                bn3 = wtile(d_bn3, [1, MN], F32)
                ones128 = wtile(d_ones, [1, 128], F32)

            # persistent state: customer-MLP output, GRU hidden states
            # cust_out layout: partition = (n%4)*32 + f, free = (n//4)*S + s
            cust = sp.tile([128, NG * S], F16, tag="cust_out")
            h1 = sp.tile([GH, S], F16, tag="h1")
            h2 = sp.tile([GH, S], F16, tag="h2")

          # timing-calibration repeat loop (reps=1 in production)
          # fmt: off
            for _rep in range(reps):
              nc.vector.memset(h1[:], 0.0)
              nc.gpsimd.memset(h2[:], 0.0)

              # ---- phase A: customer MLP (gap filler) ----------------------
              # p1 pre-activations for node pairs share a przb 2-bank tile;
              # one tanh covers both (bc1 is per-partition so this also
              # holds in the non-zb fallback).
              xtiles = {}
              def emitA(g, lowp, sbs=None):
                with (lowprio() if lowp else contextlib.nullcontext()):
                  if g not in xtiles:
                      xg = xp.tile([FEAT, 4 * S], F16, tag="xc", name=f"xg{g}")
                      with (tc.high_priority() if g == 0
                            else contextlib.nullcontext()):
                          nc.sync.dma_start(
                              xg[:], d_cust.ap()[:, 4 * g * S : 4 * (g + 1) * S]
                          )
                      xtiles[g] = xg
                  xg = xtiles[g]
                  for sb in (range(NCH) if sbs is None else sbs):
                      # stage 1: h1c for the 4 nodes (2 przb borrows, one at
                      # a time); stage 2: c2 accumulation (1 pi borrow).
                      h1cbs = []
                      for kp in range(2):
                          p1b = ps2.tile([CH, 2 * NC], F32, tag="przb",
                                         name=f"p1b_{g}_{sb}_{kp}")
                          h1cb = h1p.tile([CH, 2 * NC], F16, tag="h1c")
                          for kk in range(2):
                              k = 2 * kp + kk
                              nc.tensor.matmul(
                                  p1b[:, kk * NC : (kk + 1) * NC], wc1,
                                  xg[:, k * S + sb * NC : k * S + (sb + 1) * NC],
                              )
                          if zb:
                              nc.scalar.activation(h1cb[:], p1b[:], AF.Tanh)
                          else:
                              nc.scalar.activation(h1cb[:], p1b[:], AF.Tanh, bias=bc1[:])
                          h1cbs.append(h1cb)
                      c2 = ps1.tile([128, NC], F32, tag="pi", name=f"c2_{g}_{sb}")
                      for k in range(4):
                          nc.tensor.matmul(
                              c2[32 * k : 32 * (k + 1), :], wc2,
                              h1cbs[k // 2][:, (k % 2) * NC : (k % 2 + 1) * NC],
                              tile_position=(0, 32 * k),
                          )
                      if zb:
                          nc.scalar.activation(
                              cust[:, g * S + sb * NC : g * S + (sb + 1) * NC],
                              c2[:], AF.Tanh,
                          )
                      else:
                          nc.scalar.activation(
                              cust[:, g * S + sb * NC : g * S + (sb + 1) * NC],
                              c2[:], AF.Tanh, bias=bc2[:],
                          )

              # ---- phase B: 2-layer GRU over MN steps -----------------------
              def emitB_layer(t, layer, h, kq):
                  """One GRU layer update for step t on hidden h [GH, S]."""
                  g = t // 4
                  for c in range(NCH):
                      c0, c1 = c * NC, (c + 1) * NC
                      hc = h[:, c0:c1]
                      przb = ps2.tile([GH, 2 * NC], F32, tag="przb")
                      pr = przb[:, 0:NC]
                      pz = przb[:, NC : 2 * NC]
                      ph = ps1.tile([GH, NC], F32, tag="ph")
                      pi = ps1.tile([GH, NC], F32, tag="pi")
                      if kq is not None:
                          tp = (32 * kq, 0)
                          xc = cust[32 * kq : 32 * kq + CO, g * S + c0 : g * S + c1]
                          wx = {gg: wih0g[(kq, gg)] for gg in range(3)}
                      else:
                          tp = None
                          xc = h1[:, c0:c1]
                          wx = wih1g
                      mmkw = {} if tp is None else {"tile_position": tp}
                      nc.tensor.matmul(pr, whh[(layer, 0)], hc, start=True, stop=False)
                      nc.tensor.matmul(pr, wx[0], xc, start=False, stop=True, **mmkw)
                      nc.tensor.matmul(pz, whh[(layer, 1)], hc, start=True, stop=False)
                      nc.tensor.matmul(pz, wx[1], xc, start=False, stop=True, **mmkw)
                      nc.tensor.matmul(ph[:], whh[(layer, 2)], hc)
                      rz = wk.tile([GH, 2 * NC], F16, tag="rz")
                      if zb:
                          nc.scalar.activation(rz[:], przb[:], AF.Sigmoid)
                      else:
                          nc.scalar.activation(rz[:, 0:NC], pr, AF.Sigmoid,
                                               bias=gb[(layer, "r")][:])
                          nc.scalar.activation(rz[:, NC : 2 * NC], pz, AF.Sigmoid,
                                               bias=gb[(layer, "z")][:])
                      # the t_ -> inject chain feeds the next tanh, which is
                      # the ACT critical path; lift it over queued d_/e_/h'
                      # work from earlier cells.
                      t_c = wk.tile([GH, NC], F16, tag="t_")
                      with tc.high_priority(offset=64):
                          if zb:
                              nc.vector.tensor_mul(t_c[:], ph[:], rz[:, 0:NC])
                          else:
                              nc.vector.scalar_tensor_tensor(
                                  t_c[:], ph[:], gb[(layer, "hn")][:], rz[:, 0:NC],
                                  OP.add, OP.mult,
                              )
                          # n-gate input finishes inside PSUM: pi = I@t_ + Wih_n@x
                          nc.tensor.matmul(pi[:], ident, t_c[:], start=True, stop=False)
                          nc.tensor.matmul(pi[:], wx[2], xc, start=False, stop=True, **mmkw)
                      n_c = wk.tile([GH, NC], F16, tag="n")
                      if zb:
                          nc.scalar.activation(n_c[:], pi[:], AF.Tanh)
                      else:
                          nc.scalar.activation(n_c[:], pi[:], AF.Tanh,
                                               bias=gb[(layer, "in")][:])
                      # h' = n + z*(h - n), all DVE at fp16 2x
                      d_c = wk.tile([GH, NC], F16, tag="d_")
                      nc.vector.tensor_sub(d_c[:], hc, n_c[:])
                      e_c = wk.tile([GH, NC], F16, tag="e_")
                      nc.vector.tensor_mul(e_c[:], rz[:, NC : 2 * NC], d_c[:])
                      nc.vector.tensor_add(hc, n_c[:], e_c[:])

              # customer MLP runs one 4-node group AHEAD of the GRU, spread
              # one 512-token chunk per step so every step has ACT filler
              # work and a group boundary never stalls L1.
              emitA(0, lowp=False)
              for t in range(MN):
                  g_next, sb = t // 4 + 1, t % 4
                  if g_next < NG and sb < NCH:
                      emitA(g_next, lowp=True, sbs=[sb])
                  emitB_layer(t, 0, h1, t % 4)
                  emitB_layer(t, 1, h2, None)

              # ---- phase C: route mean + node MLP + masked softmax ----------
              mean32 = fp_.tile([GH, BLOC], F32, tag="mean32")
              h2v = h2[:].rearrange("p (b r) -> p b r", r=MR)
              nc.vector.tensor_reduce(mean32[:], h2v, mybir.AxisListType.X, OP.add)
              mean = fp_.tile([GH, BLOC], F16, tag="mean")
              nc.vector.tensor_copy(mean[:], mean32[:])
              pmt = ps1.tile([BLOC, 256], F32, tag="ph", name="cpmt")
              nc.tensor.matmul(pmt[:], mean[:], wn1b)
              mmt = fp_.tile([BLOC, 256], F16, tag="mmt")
              nc.vector.tensor_copy(mmt[:], pmt[:])

              for c in range(NCH):
                  c0, c1 = c * NC, (c + 1) * NC
                  n1 = []
                  for m in range(2):
                      p1 = ps2.tile([128, NC], F32, tag="przb", name=f"cp1_{c}_{m}")
                      nc.tensor.matmul(
                          p1[:], wn1a[m], h2[:, c0:c1], start=True, stop=False,
                      )
                      nc.tensor.matmul(
                          p1[:], mmt[:, 128 * m : 128 * (m + 1)], selc[c],
                          start=False, stop=True,
                      )
                      a1 = fp_.tile([128, NC], F16, tag=f"n1_{m}")
                      if zb:
                          nc.scalar.activation(a1[:], p1[:], AF.Relu)
                      else:
                          nc.scalar.activation(a1[:], p1[:], AF.Relu,
                                               bias=bn1[:, m : m + 1])
                      n1.append(a1)
                  p2 = ps1.tile([128, NC], F32, tag="ph", name=f"cp2_{c}")
                  nc.tensor.matmul(p2[:], wn2a, n1[0][:], start=True, stop=False)
                  nc.tensor.matmul(p2[:], wn2b, n1[1][:], start=False, stop=True)
                  n2 = fp_.tile([128, NC], F16, tag="n2")
                  if zb:
                      nc.scalar.activation(n2[:], p2[:], AF.Relu)
                  else:
                      nc.scalar.activation(n2[:], p2[:], AF.Relu, bias=bn2[:])
                  po = fp_.tile([128, 4 * MN], F32, tag="po")
                  for q in range(NC // 128):
                      tok0 = c0 + q * 128
                      pl = ps1.tile([128, MN], F32, tag="pi", name=f"cpl_{c}_{q}")
                      if zb:
                          nc.tensor.matmul(pl[:], n2[:, q * 128 : (q + 1) * 128], wn3)
                      else:
                          nc.tensor.matmul(
                              pl[:], n2[:, q * 128 : (q + 1) * 128], wn3,
                              start=True, stop=False,
                          )
                          nc.tensor.matmul(pl[:], ones128[:], bn3[:],
                                           start=False, stop=True)
                      ex = fp_.tile([128, MN], F32, tag="ex")
                      sm = fp_.tile([128, 1], F32, tag="sm")
                      nc.scalar.activation(ex[:], pl[:], AF.Exp, accum_out=sm[:])
                      rec = fp_.tile([128, 1], F32, tag="rec")
                      nc.vector.reciprocal(rec[:], sm[:])
                      msk = fp_.tile([128, MN], F32, tag="msk")
                      nc.vector.tensor_scalar(
                          msk[:], iota24, rnq[tok0 // 128], None, OP.is_lt
                      )
                      nc.vector.scalar_tensor_tensor(
                          po[:, q * MN : (q + 1) * MN], ex[:], rec[:], msk[:],
                          OP.mult, OP.mult
                      )
                  nc.sync.dma_start(
                      d_out.ap()[:, c * 4 * MN : (c + 1) * 4 * MN], po[:]
                  )

    nc.compile()
    return nc


def _prep_inputs(inputs, zb):
    """Host-side preprocessing -> list of per-core input dicts."""
    state = np.ascontiguousarray(inputs["state"], dtype=np.float32)
    rn = state[:, :MR]                                    # [B, 48]
    cust = state[:, MR:].reshape(B, MR, MN, FEAT)

    def f32(x):
        return np.ascontiguousarray(np.asarray(x, dtype=np.float32))

    Wih0 = f32(inputs["Wih0"]); Whh0 = f32(inputs["Whh0"])
    Wih1 = f32(inputs["Wih1"]); Whh1 = f32(inputs["Whh1"])

    sel = np.zeros((BLOC, S), np.float32)
    sel[np.arange(S) // MR, np.arange(S)] = 1.0

    w16v = {
        "wc1": np.asarray(inputs["Wc1"], np.float16),
        "wc2": np.asarray(inputs["Wc2"], np.float16),
        "ident": np.eye(128, dtype=np.float16),
        "wih0": np.tile(Wih0.astype(np.float16), (4, 1)),
        "whh0": Whh0.astype(np.float16),
        "wih1": Wih1.astype(np.float16),
        "whh1": Whh1.astype(np.float16),
        "wn1a": f32(inputs["Wn1"])[0:GH, :].astype(np.float16),
        "wn1b": (f32(inputs["Wn1"])[GH:, :] / np.float32(MR)).astype(np.float16),
        "wn2a": f32(inputs["Wn2"])[0:128, :].astype(np.float16),
        "wn2b": f32(inputs["Wn2"])[128:256, :].astype(np.float16),
        "wn3": np.asarray(inputs["Wn3"], np.float16),
        "sel": sel.astype(np.float16),
    }
    wpk16 = np.zeros((128, COLS16), np.float16)
    for name, (off, ncol, nrow) in _P16.items():
        v = w16v[name]
        assert v.shape == (nrow, ncol), (name, v.shape, (nrow, ncol))
        wpk16[:nrow, off : off + ncol] = v

    wmini = np.zeros((128, CH + CO), np.float16)
    wmini[:FEAT, 0:CH] = w16v["wc1"]
    wmini[:, CH : CH + CO] = w16v["wc2"]
    com = {"wpk16": wpk16, "wmini": wmini}
    if not zb:
        bih0 = f32(inputs["bih0"]); bhh0 = f32(inputs["bhh0"])
        bih1 = f32(inputs["bih1"]); bhh1 = f32(inputs["bhh1"])
        com.update({
            "bc1": f32(inputs["bc1"]).reshape(CH, 1),
            "bc2s": np.tile(f32(inputs["bc2"]).reshape(CO), 4).reshape(128, 1),
            "b0_r": (bih0[0:GH] + bhh0[0:GH]).reshape(GH, 1),
            "b0_z": (bih0[GH : 2 * GH] + bhh0[GH : 2 * GH]).reshape(GH, 1),
            "b0_in": bih0[2 * GH :].reshape(GH, 1),
            "b0_hn": bhh0[2 * GH :].reshape(GH, 1),
            "b1_r": (bih1[0:GH] + bhh1[0:GH]).reshape(GH, 1),
            "b1_z": (bih1[GH : 2 * GH] + bhh1[GH : 2 * GH]).reshape(GH, 1),
            "b1_in": bih1[2 * GH :].reshape(GH, 1),
            "b1_hn": bhh1[2 * GH :].reshape(GH, 1),
            "bn1c": np.ascontiguousarray(f32(inputs["bn1"]).reshape(2, 128).T),
            "bn2c": f32(inputs["bn2"]).reshape(128, 1),
            "bn3r": f32(inputs["bn3"]).reshape(1, MN),
            "ones128": np.ones((1, 128), np.float32),
        })

    in_maps = []
    for core in range(NCORES):
        b0, b1 = core * BLOC, (core + 1) * BLOC
        # cust_fm[f, n*S + (b*MR+r)] = cust[b, r, n, f]
        cfm = cust[b0:b1].transpose(3, 2, 0, 1).reshape(FEAT, MN * S)
        m = dict(com)
        m["cust_fm"] = np.ascontiguousarray(cfm.astype(np.float16))
        cpk32 = np.zeros((128, COLS32), np.float32)
        cpk32[:, 0:MN] = np.arange(MN, dtype=np.float32)[None, :]
        # rn_pm[p, q] = route_num of token q*128+p
        cpk32[:, MN:] = rn[b0:b1].reshape(S).reshape(NQ, 128).T
        m["cpk32"] = cpk32
        in_maps.append(m)
    return in_maps


def _zb(inputs):
    return all(
        float(np.abs(np.asarray(inputs[k], np.float32)).max()) == 0.0
        for k in ("bc1", "bc2", "bih0", "bhh0", "bih1", "bhh1",
                  "bn1", "bn2", "bn3")
    )


def _run(inputs, **kw):
    zb = _zb(inputs)
    key = ("nc", zb)
    if key not in _cache:
        _cache[key] = _build(zb=zb)
    nc = _cache[key]
    _cache["nc"] = nc  # for test harness introspection
    in_maps = _prep_inputs(inputs, zb)
    return run_bass_kernel_spmd(nc, in_maps, core_ids=list(range(NCORES)), **kw)


def kernel(**inputs) -> np.ndarray:
    res = _run(inputs)
    outs = []
    for r in res.results:
        pm = r["out_pm"]                                  # [128, NQ*MN]
        outs.append(pm.reshape(128, NQ, MN).transpose(1, 0, 2).reshape(S, MN))
    return np.concatenate(outs, axis=0).reshape(B, MR, MN)


# revision 44
# speedup vs baseline: 1.0041x; 1.0041x over previous
"""Trainium2 Bass kernel for nn_MLP_Route_RL_Model (route RL model).

Reference math (per batch element b of 256):
  - state = [route_nums (48) | customers (48*24*36)]
  - customer MLP (tanh-tanh, 36->128->32) on every node of every route
  - 2-layer GRU (hidden 128) over the 24 nodes of each of the 48 routes
  - route summary mean, node-selection MLP 256->256->128->24, masked softmax

Sharding: pure data parallel over batch B=256 -> 8 cores x 32.

Layout: feature-major activations ([feature, token] in SBUF) so matmuls
contract over the partition dim without transposes.

Schedule notes (the kernel is ACT elementwise-bound; engine cost is
free-dim elements only, so fewer/bigger ACT ops win):
  - r|z gate pre-activations share one 2-bank PSUM tile [128,1024]; with
    the (all-zero) biases dropped, ONE sigmoid covers both gates.
  - n-gate input finishes inside PSUM: after the DVE computes
    t_ = r*ph, an identity matmul accumulates I @ t_ onto the x-side
    matmul in the pi bank, deleting the separate s_ = pi + t_ DVE add.
  - h' = n + z*(h-n): 3 DVE ops/chunk at fp16 2x.
  - PSUM tags: przb (2 banks x2 slots) + ph (x2) + pi (x2) = 8 banks;
    the customer MLP borrows przb/pi slots at low scheduler priority and
    runs one 4-node group ahead of the GRU.
  - All fp16 weights ship in ONE packed DMA (SBUF views), fp32 consts in
    another; cust loads are per-group; outputs merge to one DMA per
    512-token chunk. DMA issue occupies the SP sequencer 565ns each, so
    instruction count matters.
  - A non-zero-bias fallback keeps split sigmoids + bias ports.
"""

import contextlib
import sys

import numpy as np

sys.path.insert(0, "/opt/trn_rl_repo")

import concourse.bass as bass  # noqa: E402
import concourse.bacc as bacc  # noqa: E402
import concourse.mybir as mybir  # noqa: E402
import concourse.tile as tile  # noqa: E402
from concourse.bass_utils import run_bass_kernel_spmd  # noqa: E402

F32 = mybir.dt.float32
F16 = mybir.dt.float16
AF = mybir.ActivationFunctionType
OP = mybir.AluOpType

# Problem shape constants
B = 256
NCORES = 8
BLOC = B // NCORES          # 32 batch rows per core
MR = 48                     # routes per batch
MN = 24                     # nodes per route
FEAT = 36
CH = 128                    # customer hidden
CO = 32                     # customer out
GH = 128                    # GRU hidden
S = BLOC * MR               # sequences per core = 1536
NC = 512                    # token chunk (PSUM bank width in fp32)
NCH = S // NC               # chunks per core = 3
NG = MN // 4                # node groups of 4 (cust_out partition stacking)
NQ = S // 128               # 128-token groups per core = 12

# fp16 weight pack layout: (col_offset, n_cols, n_rows)
_P16 = {}
_c = 0
for _name, _ncol, _nrow in [
    ("wc1", CH, FEAT), ("wc2", CO, CH), ("ident", 128, 128),
    ("wih0", 3 * GH, 128), ("whh0", 3 * GH, GH),
    ("wih1", 3 * GH, GH), ("whh1", 3 * GH, GH),
    ("wn1a", 256, GH), ("wn1b", 256, GH),
    ("wn2a", 128, 128), ("wn2b", 128, 128), ("wn3", MN, GH),
    ("sel", S, BLOC),
]:
    _P16[_name] = (_c, _ncol, _nrow)
    _c += _ncol
COLS16 = _c
# fp32 const pack layout
_P32 = {"iota24": (0, MN, 128), "rn_pm": (MN, NQ, 128)}
COLS32 = MN + NQ

_cache = {}


def _build(reps=1, zb=True):
    """Trace + schedule the per-core Tile kernel. zb: all biases are zero."""
    nc = bacc.Bacc("TRN2", target_bir_lowering=False, debug=False)

    # ---- DRAM I/O ----------------------------------------------------------
    d_cust = nc.dram_tensor("cust_fm", [FEAT, MN * S], F16, kind="ExternalInput")
    d_wmini = nc.dram_tensor("wmini", [128, CH + CO], F16, kind="ExternalInput")
    d_w16 = nc.dram_tensor("wpk16", [128, COLS16], F16, kind="ExternalInput")
    d_c32 = nc.dram_tensor("cpk32", [128, COLS32], F32, kind="ExternalInput")
    if not zb:
        d_bc1 = nc.dram_tensor("bc1", [CH, 1], F32, kind="ExternalInput")
        d_bc2 = nc.dram_tensor("bc2s", [128, 1], F32, kind="ExternalInput")
        d_gb = {}
        for layer in (0, 1):
            for g in ("r", "z", "in", "hn"):
                d_gb[(layer, g)] = nc.dram_tensor(
                    f"b{layer}_{g}", [GH, 1], F32, kind="ExternalInput"
                )
        d_bn1 = nc.dram_tensor("bn1c", [128, 2], F32, kind="ExternalInput")
        d_bn2 = nc.dram_tensor("bn2c", [128, 1], F32, kind="ExternalInput")
        d_bn3 = nc.dram_tensor("bn3r", [1, MN], F32, kind="ExternalInput")
        d_ones = nc.dram_tensor("ones128", [1, 128], F32, kind="ExternalInput")
    # partition-major output: out_pm[p, q*MN+m] = probs for token q*128+p
    d_out = nc.dram_tensor("out_pm", [128, NQ * MN], F32, kind="ExternalOutput")

    with tile.TileContext(nc) as tc:
        with (
            tc.tile_pool(name="wpool", bufs=1) as wp,
            tc.tile_pool(name="state", bufs=1) as sp,
            tc.tile_pool(name="xin", bufs=3) as xp,
            tc.tile_pool(name="h1c", bufs=8) as h1p,
            tc.tile_pool(name="wk", bufs=8) as wk,
            tc.tile_pool(name="fin", bufs=4) as fp_,
            tc.tile_pool(name="ps2", bufs=2, space="PSUM") as ps2,
            tc.tile_pool(name="ps1", bufs=2, space="PSUM") as ps1,
        ):
            def lowprio():
                # deprioritize: scheduler runs these only in recurrence gaps
                return tc.high_priority(offset=-1_000_000)

            # ---- packed weights / constants ---------------------------------
            # customer-MLP weights ship in a small DMA ahead of the big pack
            # so phase A's first matmul isn't gated on the full 1MB transfer.
            wmini = wp.tile([128, CH + CO], F16, tag="wmini")
            nc.sync.dma_start(wmini[:], d_wmini.ap())
            wpk = wp.tile([128, COLS16], F16, tag="wpk16")
            nc.sync.dma_start(wpk[:], d_w16.ap())
            cpk = wp.tile([128, COLS32], F32, tag="cpk32")
            nc.sync.dma_start(cpk[:], d_c32.ap())

            def w16(name, r0=0, rows=None, c0=0, cols=None):
                off, ncol, nrow = _P16[name]
                rows = nrow if rows is None else rows
                cols = ncol if cols is None else cols
                return wpk[r0 : r0 + rows, off + c0 : off + c0 + cols]

            wc1 = wmini[0:FEAT, 0:CH]
            wc2 = wmini[0:CH, CH : CH + CO]
            ident = w16("ident")
            # GRU weight gate slices; L1 x-side per partition-quadrant k
            whh = {
                (0, g): w16("whh0", c0=g * GH, cols=GH) for g in range(3)
            }
            whh.update({
                (1, g): w16("whh1", c0=g * GH, cols=GH) for g in range(3)
            })
            wih1g = {g: w16("wih1", c0=g * GH, cols=GH) for g in range(3)}
            wih0g = {
                (k, g): w16("wih0", r0=32 * k, rows=CO, c0=g * GH, cols=GH)
                for k in range(4) for g in range(3)
            }
            wn1a = {m: w16("wn1a", c0=128 * m, cols=128) for m in range(2)}
            wn1b = w16("wn1b")
            wn2a = w16("wn2a")
            wn2b = w16("wn2b")
            wn3 = w16("wn3")
            selc = {c: w16("sel", c0=c * NC, cols=NC) for c in range(NCH)}
            iota24 = cpk[0:128, 0:MN]
            rnq = {q: cpk[0:128, MN + q : MN + q + 1] for q in range(NQ)}

            gb = {}
            if not zb:
                def wtile(dram, shape, dtype):
                    t = wp.tile(shape, dtype, tag=dram.name)
                    nc.sync.dma_start(t[:], dram.ap())
                    return t
                bc1 = wtile(d_bc1, [CH, 1], F32)
                bc2 = wtile(d_bc2, [128, 1], F32)
                for k, d in d_gb.items():
                    gb[k] = wtile(d, [GH, 1], F32)
                bn1 = wtile(d_bn1, [128, 2], F32)
                bn2 = wtile(d_bn2, [128, 1], F32)
                bn3 = wtile(d_bn3, [1, MN], F32)
                ones128 = wtile(d_ones, [1, 128], F32)

            # persistent state: customer-MLP output, GRU hidden states
            # cust_out layout: partition = (n%4)*32 + f, free = (n//4)*S + s
            cust = sp.tile([128, NG * S], F16, tag="cust_out")
            h1 = sp.tile([GH, S], F16, tag="h1")
            h2 = sp.tile([GH, S], F16, tag="h2")

          # timing-calibration repeat loop (reps=1 in production)
          # fmt: off
            for _rep in range(reps):
              nc.vector.memset(h1[:], 0.0)
              nc.gpsimd.memset(h2[:], 0.0)

              # ---- phase A: customer MLP (gap filler) ----------------------
              # p1 pre-activations for node pairs share a przb 2-bank tile;
              # one tanh covers both (bc1 is per-partition so this also
              # holds in the non-zb fallback).
              xtiles = {}
              def emitA(g, lowp, sbs=None):
                with (lowprio() if lowp else contextlib.nullcontext()):
                  if g not in xtiles:
                      xg = xp.tile([FEAT, 4 * S], F16, tag="xc", name=f"xg{g}")
                      with (tc.high_priority() if g == 0
                            else contextlib.nullcontext()):
                          nc.sync.dma_start(
                              xg[:], d_cust.ap()[:, 4 * g * S : 4 * (g + 1) * S]
                          )
                      xtiles[g] = xg
                  xg = xtiles[g]
                  for sb in (range(NCH) if sbs is None else sbs):
                      # stage 1: h1c for the 4 nodes (2 przb borrows, one at
                      # a time); stage 2: c2 accumulation (1 pi borrow).
                      h1cbs = []
                      for kp in range(2):
                          p1b = ps2.tile([CH, 2 * NC], F32, tag="przb",
                                         name=f"p1b_{g}_{sb}_{kp}")
                          h1cb = h1p.tile([CH, 2 * NC], F16, tag="h1c")
                          for kk in range(2):
                              k = 2 * kp + kk
                              nc.tensor.matmul(
                                  p1b[:, kk * NC : (kk + 1) * NC], wc1,
                                  xg[:, k * S + sb * NC : k * S + (sb + 1) * NC],
                              )
                          if zb:
                              nc.scalar.activation(h1cb[:], p1b[:], AF.Tanh)
                          else:
                              nc.scalar.activation(h1cb[:], p1b[:], AF.Tanh, bias=bc1[:])
                          h1cbs.append(h1cb)
                      c2 = ps1.tile([128, NC], F32, tag="pi", name=f"c2_{g}_{sb}")
                      for k in range(4):
                          nc.tensor.matmul(
                              c2[32 * k : 32 * (k + 1), :], wc2,
                              h1cbs[k // 2][:, (k % 2) * NC : (k % 2 + 1) * NC],
                              tile_position=(0, 32 * k),
                          )
                      if zb:
                          nc.scalar.activation(
                              cust[:, g * S + sb * NC : g * S + (sb + 1) * NC],
                              c2[:], AF.Tanh,
                          )
                      else:
                          nc.scalar.activation(
                              cust[:, g * S + sb * NC : g * S + (sb + 1) * NC],
                              c2[:], AF.Tanh, bias=bc2[:],
                          )

              # ---- phase B: 2-layer GRU over MN steps -----------------------
              def emitB_layer(t, layer, h, kq):
                  """One GRU layer update for step t on hidden h [GH, S]."""
                  g = t // 4
                  for c in range(NCH):
                      c0, c1 = c * NC, (c + 1) * NC
                      hc = h[:, c0:c1]
                      przb = ps2.tile([GH, 2 * NC], F32, tag="przb")
                      pr = przb[:, 0:NC]
                      pz = przb[:, NC : 2 * NC]
                      ph = ps1.tile([GH, NC], F32, tag="ph")
                      pi = ps1.tile([GH, NC], F32, tag="pi")
                      if kq is not None:
                          tp = (32 * kq, 0)
                          xc = cust[32 * kq : 32 * kq + CO, g * S + c0 : g * S + c1]
                          wx = {gg: wih0g[(kq, gg)] for gg in range(3)}
                      else:
                          tp = None
                          xc = h1[:, c0:c1]
                          wx = wih1g
                      mmkw = {} if tp is None else {"tile_position": tp}
                      nc.tensor.matmul(pr, whh[(layer, 0)], hc, start=True, stop=False)
                      nc.tensor.matmul(pr, wx[0], xc, start=False, stop=True, **mmkw)
                      nc.tensor.matmul(pz, whh[(layer, 1)], hc, start=True, stop=False)
                      nc.tensor.matmul(pz, wx[1], xc, start=False, stop=True, **mmkw)
                      nc.tensor.matmul(ph[:], whh[(layer, 2)], hc)
                      rz = wk.tile([GH, 2 * NC], F16, tag="rz")
                      if zb:
                          nc.scalar.activation(rz[:], przb[:], AF.Sigmoid)
                      else:
                          nc.scalar.activation(rz[:, 0:NC], pr, AF.Sigmoid,
                                               bias=gb[(layer, "r")][:])
                          nc.scalar.activation(rz[:, NC : 2 * NC], pz, AF.Sigmoid,
                                               bias=gb[(layer, "z")][:])
                      # the t_ -> inject chain feeds the next tanh, which is
                      # the ACT critical path; lift it over queued d_/e_/h'
                      # work from earlier cells.
                      t_c = wk.tile([GH, NC], F16, tag="t_")
                      with tc.high_priority(offset=64):
                          if zb:
                              nc.vector.tensor_mul(t_c[:], ph[:], rz[:, 0:NC])
                          else:
                              nc.vector.scalar_tensor_tensor(
                                  t_c[:], ph[:], gb[(layer, "hn")][:], rz[:, 0:NC],
                                  OP.add, OP.mult,
                              )
                          # n-gate input finishes inside PSUM: pi = I@t_ + Wih_n@x
                          nc.tensor.matmul(pi[:], ident, t_c[:], start=True, stop=False)
                          nc.tensor.matmul(pi[:], wx[2], xc, start=False, stop=True, **mmkw)
                      n_c = wk.tile([GH, NC], F16, tag="n")
                      if zb:
                          nc.scalar.activation(n_c[:], pi[:], AF.Tanh)
                      else:
                          nc.scalar.activation(n_c[:], pi[:], AF.Tanh,
                                               bias=gb[(layer, "in")][:])
                      # h' = n + z*(h - n), all DVE at fp16 2x
                      d_c = wk.tile([GH, NC], F16, tag="d_")
                      nc.vector.tensor_sub(d_c[:], hc, n_c[:])
                      e_c = wk.tile([GH, NC], F16, tag="e_")
                      nc.vector.tensor_mul(e_c[:], rz[:, NC : 2 * NC], d_c[:])
                      nc.vector.tensor_add(hc, n_c[:], e_c[:])

              # customer MLP runs one 4-node group AHEAD of the GRU, spread
              # one 512-token chunk per step so every step has ACT filler
              # work and a group boundary never stalls L1.
              emitA(0, lowp=False)
              for t in range(MN):
                  g_next, sb = t // 4 + 1, t % 4
                  if g_next < NG and sb < NCH:
                      emitA(g_next, lowp=True, sbs=[sb])
                  emitB_layer(t, 0, h1, t % 4)
                  emitB_layer(t, 1, h2, None)

              # ---- phase C: route mean + node MLP + masked softmax ----------
              mean32 = fp_.tile([GH, BLOC], F32, tag="mean32")
              h2v = h2[:].rearrange("p (b r) -> p b r", r=MR)
              nc.vector.tensor_reduce(mean32[:], h2v, mybir.AxisListType.X, OP.add)
              mean = fp_.tile([GH, BLOC], F16, tag="mean")
              nc.vector.tensor_copy(mean[:], mean32[:])
              pmt = ps1.tile([BLOC, 256], F32, tag="ph", name="cpmt")
              nc.tensor.matmul(pmt[:], mean[:], wn1b)
              mmt = fp_.tile([BLOC, 256], F16, tag="mmt")
              nc.vector.tensor_copy(mmt[:], pmt[:])

              for c in range(NCH):
                  c0, c1 = c * NC, (c + 1) * NC
                  n1 = []
                  for m in range(2):
                      p1 = ps2.tile([128, NC], F32, tag="przb", name=f"cp1_{c}_{m}")
                      nc.tensor.matmul(
                          p1[:], wn1a[m], h2[:, c0:c1], start=True, stop=False,
                      )
                      nc.tensor.matmul(
                          p1[:], mmt[:, 128 * m : 128 * (m + 1)], selc[c],
                          start=False, stop=True,
                      )
                      a1 = fp_.tile([128, NC], F16, tag=f"n1_{m}")
                      if zb:
                          nc.scalar.activation(a1[:], p1[:], AF.Relu)
                      else:
                          nc.scalar.activation(a1[:], p1[:], AF.Relu,
                                               bias=bn1[:, m : m + 1])
                      n1.append(a1)
                  p2 = ps1.tile([128, NC], F32, tag="ph", name=f"cp2_{c}")
                  nc.tensor.matmul(p2[:], wn2a, n1[0][:], start=True, stop=False)
                  nc.tensor.matmul(p2[:], wn2b, n1[1][:], start=False, stop=True)
                  n2 = fp_.tile([128, NC], F16, tag="n2")
                  if zb:
                      nc.scalar.activation(n2[:], p2[:], AF.Relu)
                  else:
                      nc.scalar.activation(n2[:], p2[:], AF.Relu, bias=bn2[:])
                  po = fp_.tile([128, 4 * MN], F32, tag="po")
                  for q in range(NC // 128):
                      tok0 = c0 + q * 128
                      pl = ps1.tile([128, MN], F32, tag="pi", name=f"cpl_{c}_{q}")
                      if zb:
                          nc.tensor.matmul(pl[:], n2[:, q * 128 : (q + 1) * 128], wn3)
                      else:
                          nc.tensor.matmul(
                              pl[:], n2[:, q * 128 : (q + 1) * 128], wn3,
                              start=True, stop=False,
                          )
                          nc.tensor.matmul(pl[:], ones128[:], bn3[:],
                                           start=False, stop=True)
                      ex = fp_.tile([128, MN], F32, tag="ex")
                      sm = fp_.tile([128, 1], F32, tag="sm")
                      nc.scalar.activation(ex[:], pl[:], AF.Exp, accum_out=sm[:])
                      rec = fp_.tile([128, 1], F32, tag="rec")
                      nc.vector.reciprocal(rec[:], sm[:])
                      msk = fp_.tile([128, MN], F32, tag="msk")
                      nc.vector.tensor_scalar(
                          msk[:], iota24, rnq[tok0 // 128], None, OP.is_lt
                      )
                      nc.vector.scalar_tensor_tensor(
                          po[:, q * MN : (q + 1) * MN], ex[:], rec[:], msk[:],
                          OP.mult, OP.mult
                      )
                  nc.sync.dma_start(
                      d_out.ap()[:, c * 4 * MN : (c + 1) * 4 * MN], po[:]
                  )

    nc.compile()
    return nc


def _prep_inputs(inputs, zb):
    """Host-side preprocessing -> list of per-core input dicts."""
    state = np.ascontiguousarray(inputs["state"], dtype=np.float32)
    rn = state[:, :MR]                                    # [B, 48]
    cust = state[:, MR:].reshape(B, MR, MN, FEAT)

    def f32(x):
        return np.ascontiguousarray(np.asarray(x, dtype=np.float32))

    Wih0 = f32(inputs["Wih0"]); Whh0 = f32(inputs["Whh0"])
    Wih1 = f32(inputs["Wih1"]); Whh1 = f32(inputs["Whh1"])

    sel = np.zeros((BLOC, S), np.float32)
    sel[np.arange(S) // MR, np.arange(S)] = 1.0

    w16v = {
        "wc1": np.asarray(inputs["Wc1"], np.float16),
        "wc2": np.asarray(inputs["Wc2"], np.float16),
        "ident": np.eye(128, dtype=np.float16),
        "wih0": np.tile(Wih0.astype(np.float16), (4, 1)),
        "whh0": Whh0.astype(np.float16),
        "wih1": Wih1.astype(np.float16),
        "whh1": Whh1.astype(np.float16),
        "wn1a": f32(inputs["Wn1"])[0:GH, :].astype(np.float16),
        "wn1b": (f32(inputs["Wn1"])[GH:, :] / np.float32(MR)).astype(np.float16),
        "wn2a": f32(inputs["Wn2"])[0:128, :].astype(np.float16),
        "wn2b": f32(inputs["Wn2"])[128:256, :].astype(np.float16),
        "wn3": np.asarray(inputs["Wn3"], np.float16),
        "sel": sel.astype(np.float16),
    }
    wpk16 = np.zeros((128, COLS16), np.float16)
    for name, (off, ncol, nrow) in _P16.items():
        v = w16v[name]
        assert v.shape == (nrow, ncol), (name, v.shape, (nrow, ncol))
        wpk16[:nrow, off : off + ncol] = v

    wmini = np.zeros((128, CH + CO), np.float16)
    wmini[:FEAT, 0:CH] = w16v["wc1"]
    wmini[:, CH : CH + CO] = w16v["wc2"]
    com = {"wpk16": wpk16, "wmini": wmini}
    if not zb:
        bih0 = f32(inputs["bih0"]); bhh0 = f32(inputs["bhh0"])
        bih1 = f32(inputs["bih1"]); bhh1 = f32(inputs["bhh1"])
        com.update({
            "bc1": f32(inputs["bc1"]).reshape(CH, 1),
            "bc2s": np.tile(f32(inputs["bc2"]).reshape(CO), 4).reshape(128, 1),
            "b0_r": (bih0[0:GH] + bhh0[0:GH]).reshape(GH, 1),
            "b0_z": (bih0[GH : 2 * GH] + bhh0[GH : 2 * GH]).reshape(GH, 1),
            "b0_in": bih0[2 * GH :].reshape(GH, 1),
            "b0_hn": bhh0[2 * GH :].reshape(GH, 1),
            "b1_r": (bih1[0:GH] + bhh1[0:GH]).reshape(GH, 1),
            "b1_z": (bih1[GH : 2 * GH] + bhh1[GH : 2 * GH]).reshape(GH, 1),
            "b1_in": bih1[2 * GH :].reshape(GH, 1),
            "b1_hn": bhh1[2 * GH :].reshape(GH, 1),
            "bn1c": np.ascontiguousarray(f32(inputs["bn1"]).reshape(2, 128).T),
            "bn2c": f32(inputs["bn2"]).reshape(128, 1),
            "bn3r": f32(inputs["bn3"]).reshape(1, MN),
            "ones128": np.ones((1, 128), np.float32),
        })

    in_maps = []
    for core in range(NCORES):
        b0, b1 = core * BLOC, (core + 1) * BLOC
        # cust_fm[f, n*S + (b*MR+r)] = cust[b, r, n, f]
        cfm = cust[b0:b1].transpose(3, 2, 0, 1).reshape(FEAT, MN * S)
        m = dict(com)
        m["cust_fm"] = np.ascontiguousarray(cfm.astype(np.float16))
        cpk32 = np.zeros((128, COLS32), np.float32)
        cpk32[:, 0:MN] = np.arange(MN, dtype=np.float32)[None, :]
        # rn_pm[p, q] = route_num of token q*128+p
        cpk32[:, MN:] = rn[b0:b1].reshape(S).reshape(NQ, 128).T
        m["cpk32"] = cpk32
        in_maps.append(m)
    return in_maps


def _zb(inputs):
    return all(
        float(np.abs(np.asarray(inputs[k], np.float32)).max()) == 0.0
        for k in ("bc1", "bc2", "bih0", "bhh0", "bih1", "bhh1",
                  "bn1", "bn2", "bn3")
    )


def _run(inputs, **kw):
    zb = _zb(inputs)
    key = ("nc", zb)
    if key not in _cache:
        _cache[key] = _build(zb=zb)
    nc = _cache[key]
    _cache["nc"] = nc  # for test harness introspection
    in_maps = _prep_inputs(inputs, zb)
    return run_bass_kernel_spmd(nc, in_maps, core_ids=list(range(NCORES)), **kw)


def kernel(**inputs) -> np.ndarray:
    res = _run(inputs)
    outs = []
    for r in res.results:
        pm = r["out_pm"]                                  # [128, NQ*MN]
        outs.append(pm.reshape(128, NQ, MN).transpose(1, 0, 2).reshape(S, MN))
    return np.concatenate(outs, axis=0).reshape(B, MR, MN)


# revision 46
# speedup vs baseline: 1.0046x; 1.0005x over previous
"""Trainium2 Bass kernel for nn_MLP_Route_RL_Model (route RL model).

Reference math (per batch element b of 256):
  - state = [route_nums (48) | customers (48*24*36)]
  - customer MLP (tanh-tanh, 36->128->32) on every node of every route
  - 2-layer GRU (hidden 128) over the 24 nodes of each of the 48 routes
  - route summary mean, node-selection MLP 256->256->128->24, masked softmax

Sharding: pure data parallel over batch B=256 -> 8 cores x 32.

Layout: feature-major activations ([feature, token] in SBUF) so matmuls
contract over the partition dim without transposes.

Schedule notes (the kernel is ACT elementwise-bound; engine cost is
free-dim elements only, so fewer/bigger ACT ops win):
  - r|z gate pre-activations share one 2-bank PSUM tile [128,1024]; with
    the (all-zero) biases dropped, ONE sigmoid covers both gates.
  - n-gate input finishes inside PSUM: after the DVE computes
    t_ = r*ph, an identity matmul accumulates I @ t_ onto the x-side
    matmul in the pi bank, deleting the separate s_ = pi + t_ DVE add.
  - h' = n + z*(h-n): 3 DVE ops/chunk at fp16 2x.
  - PSUM tags: przb (2 banks x2 slots) + ph (x2) + pi (x2) = 8 banks;
    the customer MLP borrows przb/pi slots at low scheduler priority and
    runs one 4-node group ahead of the GRU.
  - All fp16 weights ship in ONE packed DMA (SBUF views), fp32 consts in
    another; cust loads are per-group; outputs merge to one DMA per
    512-token chunk. DMA issue occupies the SP sequencer 565ns each, so
    instruction count matters.
  - A non-zero-bias fallback keeps split sigmoids + bias ports.
"""

import contextlib
import sys

import numpy as np

sys.path.insert(0, "/opt/trn_rl_repo")

import concourse.bass as bass  # noqa: E402
import concourse.bacc as bacc  # noqa: E402
import concourse.mybir as mybir  # noqa: E402
import concourse.tile as tile  # noqa: E402
from concourse.bass_utils import run_bass_kernel_spmd  # noqa: E402

F32 = mybir.dt.float32
F16 = mybir.dt.float16
AF = mybir.ActivationFunctionType
OP = mybir.AluOpType

# Problem shape constants
B = 256
NCORES = 8
BLOC = B // NCORES          # 32 batch rows per core
MR = 48                     # routes per batch
MN = 24                     # nodes per route
FEAT = 36
CH = 128                    # customer hidden
CO = 32                     # customer out
GH = 128                    # GRU hidden
S = BLOC * MR               # sequences per core = 1536
NC = 512                    # token chunk (PSUM bank width in fp32)
NCH = S // NC               # chunks per core = 3
NG = MN // 4                # node groups of 4 (cust_out partition stacking)
NQ = S // 128               # 128-token groups per core = 12

# fp16 weight pack layout: (col_offset, n_cols, n_rows)
_P16 = {}
_c = 0
for _name, _ncol, _nrow in [
    ("wc1", CH, FEAT), ("wc2", CO, CH), ("ident", 128, 128),
    ("wih0", 3 * GH, 128), ("whh0", 3 * GH, GH),
    ("wih1", 3 * GH, GH), ("whh1", 3 * GH, GH),
    ("wn1a", 256, GH), ("wn1b", 256, GH),
    ("wn2a", 128, 128), ("wn2b", 128, 128), ("wn3", MN, GH),
    ("sel", S, BLOC),
]:
    _P16[_name] = (_c, _ncol, _nrow)
    _c += _ncol
COLS16 = _c
# fp32 const pack layout
_P32 = {"iota24": (0, MN, 128), "rn_pm": (MN, NQ, 128)}
COLS32 = MN + NQ

_cache = {}


def _build(reps=1, zb=True):
    """Trace + schedule the per-core Tile kernel. zb: all biases are zero."""
    nc = bacc.Bacc("TRN2", target_bir_lowering=False, debug=False)

    # ---- DRAM I/O ----------------------------------------------------------
    d_cust = nc.dram_tensor("cust_fm", [FEAT, MN * S], F16, kind="ExternalInput")
    d_wmini = nc.dram_tensor("wmini", [128, CH + CO], F16, kind="ExternalInput")
    d_w16 = nc.dram_tensor("wpk16", [128, COLS16], F16, kind="ExternalInput")
    d_c32 = nc.dram_tensor("cpk32", [128, COLS32], F32, kind="ExternalInput")
    if not zb:
        d_bc1 = nc.dram_tensor("bc1", [CH, 1], F32, kind="ExternalInput")
        d_bc2 = nc.dram_tensor("bc2s", [128, 1], F32, kind="ExternalInput")
        d_gb = {}
        for layer in (0, 1):
            for g in ("r", "z", "in", "hn"):
                d_gb[(layer, g)] = nc.dram_tensor(
                    f"b{layer}_{g}", [GH, 1], F32, kind="ExternalInput"
                )
        d_bn1 = nc.dram_tensor("bn1c", [128, 2], F32, kind="ExternalInput")
        d_bn2 = nc.dram_tensor("bn2c", [128, 1], F32, kind="ExternalInput")
        d_bn3 = nc.dram_tensor("bn3r", [1, MN], F32, kind="ExternalInput")
        d_ones = nc.dram_tensor("ones128", [1, 128], F32, kind="ExternalInput")
    # partition-major output: out_pm[p, q*MN+m] = probs for token q*128+p
    d_out = nc.dram_tensor("out_pm", [128, NQ * MN], F32, kind="ExternalOutput")

    with tile.TileContext(nc) as tc:
        with (
            tc.tile_pool(name="wpool", bufs=1) as wp,
            tc.tile_pool(name="state", bufs=1) as sp,
            tc.tile_pool(name="xin", bufs=3) as xp,
            tc.tile_pool(name="h1c", bufs=8) as h1p,
            tc.tile_pool(name="wk", bufs=8) as wk,
            tc.tile_pool(name="fin", bufs=4) as fp_,
            tc.tile_pool(name="ps2", bufs=2, space="PSUM") as ps2,
            tc.tile_pool(name="ps1", bufs=2, space="PSUM") as ps1,
        ):
            def lowprio():
                # deprioritize: scheduler runs these only in recurrence gaps
                return tc.high_priority(offset=-1_000_000)

            # ---- packed weights / constants ---------------------------------
            # customer-MLP weights ship in a small DMA ahead of the big pack
            # so phase A's first matmul isn't gated on the full 1MB transfer.
            wmini = wp.tile([128, CH + CO], F16, tag="wmini")
            nc.sync.dma_start(wmini[:], d_wmini.ap())
            wpk = wp.tile([128, COLS16], F16, tag="wpk16")
            nc.sync.dma_start(wpk[:], d_w16.ap())
            cpk = wp.tile([128, COLS32], F32, tag="cpk32")
            nc.sync.dma_start(cpk[:], d_c32.ap())

            def w16(name, r0=0, rows=None, c0=0, cols=None):
                off, ncol, nrow = _P16[name]
                rows = nrow if rows is None else rows
                cols = ncol if cols is None else cols
                return wpk[r0 : r0 + rows, off + c0 : off + c0 + cols]

            wc1 = wmini[0:FEAT, 0:CH]
            wc2 = wmini[0:CH, CH : CH + CO]
            ident = w16("ident")
            # GRU weight gate slices; L1 x-side per partition-quadrant k
            whh = {
                (0, g): w16("whh0", c0=g * GH, cols=GH) for g in range(3)
            }
            whh.update({
                (1, g): w16("whh1", c0=g * GH, cols=GH) for g in range(3)
            })
            wih1g = {g: w16("wih1", c0=g * GH, cols=GH) for g in range(3)}
            wih0g = {
                (k, g): w16("wih0", r0=32 * k, rows=CO, c0=g * GH, cols=GH)
                for k in range(4) for g in range(3)
            }
            wn1a = {m: w16("wn1a", c0=128 * m, cols=128) for m in range(2)}
            wn1b = w16("wn1b")
            wn2a = w16("wn2a")
            wn2b = w16("wn2b")
            wn3 = w16("wn3")
            selc = {c: w16("sel", c0=c * NC, cols=NC) for c in range(NCH)}
            iota24 = cpk[0:128, 0:MN]
            rnq = {q: cpk[0:128, MN + q : MN + q + 1] for q in range(NQ)}

            gb = {}
            if not zb:
                def wtile(dram, shape, dtype):
                    t = wp.tile(shape, dtype, tag=dram.name)
                    nc.sync.dma_start(t[:], dram.ap())
                    return t
                bc1 = wtile(d_bc1, [CH, 1], F32)
                bc2 = wtile(d_bc2, [128, 1], F32)
                for k, d in d_gb.items():
                    gb[k] = wtile(d, [GH, 1], F32)
                bn1 = wtile(d_bn1, [128, 2], F32)
                bn2 = wtile(d_bn2, [128, 1], F32)
                bn3 = wtile(d_bn3, [1, MN], F32)
                ones128 = wtile(d_ones, [1, 128], F32)

            # persistent state: customer-MLP output, GRU hidden states
            # cust_out layout: partition = (n%4)*32 + f, free = (n//4)*S + s
            cust = sp.tile([128, NG * S], F16, tag="cust_out")
            h1 = sp.tile([GH, S], F16, tag="h1")
            h2 = sp.tile([GH, S], F16, tag="h2")

          # timing-calibration repeat loop (reps=1 in production)
          # fmt: off
            for _rep in range(reps):
              nc.vector.memset(h1[:], 0.0)
              nc.gpsimd.memset(h2[:], 0.0)

              # ---- phase A: customer MLP (gap filler) ----------------------
              # p1 pre-activations for node pairs share a przb 2-bank tile;
              # one tanh covers both (bc1 is per-partition so this also
              # holds in the non-zb fallback).
              xtiles = {}
              def emitA(g, lowp, sbs=None):
                with (lowprio() if lowp else contextlib.nullcontext()):
                  if g not in xtiles:
                      xg = xp.tile([FEAT, 4 * S], F16, tag="xc", name=f"xg{g}")
                      with (tc.high_priority() if g == 0
                            else contextlib.nullcontext()):
                          nc.sync.dma_start(
                              xg[:], d_cust.ap()[:, 4 * g * S : 4 * (g + 1) * S]
                          )
                      xtiles[g] = xg
                  xg = xtiles[g]
                  for sb in (range(NCH) if sbs is None else sbs):
                      # stage 1: h1c for the 4 nodes (2 przb borrows, one at
                      # a time); stage 2: c2 accumulation (1 pi borrow).
                      h1cbs = []
                      for kp in range(2):
                          p1b = ps2.tile([CH, 2 * NC], F32, tag="przb",
                                         name=f"p1b_{g}_{sb}_{kp}")
                          h1cb = h1p.tile([CH, 2 * NC], F16, tag="h1c")
                          for kk in range(2):
                              k = 2 * kp + kk
                              nc.tensor.matmul(
                                  p1b[:, kk * NC : (kk + 1) * NC], wc1,
                                  xg[:, k * S + sb * NC : k * S + (sb + 1) * NC],
                              )
                          if zb:
                              nc.scalar.activation(h1cb[:], p1b[:], AF.Tanh)
                          else:
                              nc.scalar.activation(h1cb[:], p1b[:], AF.Tanh, bias=bc1[:])
                          h1cbs.append(h1cb)
                      c2 = ps1.tile([128, NC], F32, tag="pi", name=f"c2_{g}_{sb}")
                      for k in range(4):
                          nc.tensor.matmul(
                              c2[32 * k : 32 * (k + 1), :], wc2,
                              h1cbs[k // 2][:, (k % 2) * NC : (k % 2 + 1) * NC],
                              tile_position=(0, 32 * k),
                          )
                      if zb:
                          nc.scalar.activation(
                              cust[:, g * S + sb * NC : g * S + (sb + 1) * NC],
                              c2[:], AF.Tanh,
                          )
                      else:
                          nc.scalar.activation(
                              cust[:, g * S + sb * NC : g * S + (sb + 1) * NC],
                              c2[:], AF.Tanh, bias=bc2[:],
                          )

              # ---- phase B: 2-layer GRU over MN steps -----------------------
              def emitB_layer(t, layer, h, kq):
                  """One GRU layer update for step t on hidden h [GH, S]."""
                  g = t // 4
                  for c in range(NCH):
                      c0, c1 = c * NC, (c + 1) * NC
                      hc = h[:, c0:c1]
                      przb = ps2.tile([GH, 2 * NC], F32, tag="przb")
                      pr = przb[:, 0:NC]
                      pz = przb[:, NC : 2 * NC]
                      ph = ps1.tile([GH, NC], F32, tag="ph")
                      pi = ps1.tile([GH, NC], F32, tag="pi")
                      if kq is not None:
                          tp = (32 * kq, 0)
                          xc = cust[32 * kq : 32 * kq + CO, g * S + c0 : g * S + c1]
                          wx = {gg: wih0g[(kq, gg)] for gg in range(3)}
                      else:
                          tp = None
                          xc = h1[:, c0:c1]
                          wx = wih1g
                      mmkw = {} if tp is None else {"tile_position": tp}
                      nc.tensor.matmul(pr, whh[(layer, 0)], hc, start=True, stop=False)
                      nc.tensor.matmul(pr, wx[0], xc, start=False, stop=True, **mmkw)
                      nc.tensor.matmul(pz, whh[(layer, 1)], hc, start=True, stop=False)
                      nc.tensor.matmul(pz, wx[1], xc, start=False, stop=True, **mmkw)
                      nc.tensor.matmul(ph[:], whh[(layer, 2)], hc)
                      rz = wk.tile([GH, 2 * NC], F16, tag="rz")
                      if zb:
                          nc.scalar.activation(rz[:], przb[:], AF.Sigmoid)
                      else:
                          nc.scalar.activation(rz[:, 0:NC], pr, AF.Sigmoid,
                                               bias=gb[(layer, "r")][:])
                          nc.scalar.activation(rz[:, NC : 2 * NC], pz, AF.Sigmoid,
                                               bias=gb[(layer, "z")][:])
                      # the t_ -> inject chain feeds the next tanh, which is
                      # the ACT critical path; lift it over queued d_/e_/h'
                      # work from earlier cells.
                      t_c = wk.tile([GH, NC], F16, tag="t_")
                      with tc.high_priority(offset=64):
                          if zb:
                              nc.vector.tensor_mul(t_c[:], ph[:], rz[:, 0:NC])
                          else:
                              nc.vector.scalar_tensor_tensor(
                                  t_c[:], ph[:], gb[(layer, "hn")][:], rz[:, 0:NC],
                                  OP.add, OP.mult,
                              )
                          # n-gate input finishes inside PSUM: pi = I@t_ + Wih_n@x
                          nc.tensor.matmul(pi[:], ident, t_c[:], start=True, stop=False)
                          nc.tensor.matmul(pi[:], wx[2], xc, start=False, stop=True, **mmkw)
                      n_c = wk.tile([GH, NC], F16, tag="n")
                      if zb:
                          nc.scalar.activation(n_c[:], pi[:], AF.Tanh)
                      else:
                          nc.scalar.activation(n_c[:], pi[:], AF.Tanh,
                                               bias=gb[(layer, "in")][:])
                      # h' = n + z*(h - n), all DVE at fp16 2x
                      d_c = wk.tile([GH, NC], F16, tag="d_")
                      nc.vector.tensor_sub(d_c[:], hc, n_c[:])
                      e_c = wk.tile([GH, NC], F16, tag="e_")
                      nc.vector.tensor_mul(e_c[:], rz[:, NC : 2 * NC], d_c[:])
                      nc.vector.tensor_add(hc, n_c[:], e_c[:])

              # customer MLP runs one 4-node group AHEAD of the GRU, spread
              # one 512-token chunk per step so every step has ACT filler
              # work and a group boundary never stalls L1.
              emitA(0, lowp=False)
              for t in range(MN):
                  g_next, sb = t // 4 + 1, t % 4
                  if g_next < NG and sb < NCH:
                      emitA(g_next, lowp=True, sbs=[sb])
                  emitB_layer(t, 0, h1, t % 4)
                  emitB_layer(t, 1, h2, None)

              # ---- phase C: route mean + node MLP + masked softmax ----------
              h2v = h2[:].rearrange("p (b r) -> p b r", r=MR)
              mean = fp_.tile([GH, BLOC], F16, tag="mean")
              with nc.allow_low_precision(reason="route-mean in fp16; rel tol 2e-2"):
                  nc.vector.tensor_reduce(mean[:], h2v, mybir.AxisListType.X, OP.add)
              pmt = ps1.tile([BLOC, 256], F32, tag="ph", name="cpmt")
              nc.tensor.matmul(pmt[:], mean[:], wn1b)
              mmt = fp_.tile([BLOC, 256], F16, tag="mmt")
              nc.vector.tensor_copy(mmt[:], pmt[:])

              for c in range(NCH):
                  c0, c1 = c * NC, (c + 1) * NC
                  n1 = []
                  for m in range(2):
                      p1 = ps2.tile([128, NC], F32, tag="przb", name=f"cp1_{c}_{m}")
                      nc.tensor.matmul(
                          p1[:], wn1a[m], h2[:, c0:c1], start=True, stop=False,
                      )
                      nc.tensor.matmul(
                          p1[:], mmt[:, 128 * m : 128 * (m + 1)], selc[c],
                          start=False, stop=True,
                      )
                      a1 = fp_.tile([128, NC], F16, tag=f"n1_{m}")
                      if zb:
                          nc.scalar.activation(a1[:], p1[:], AF.Relu)
                      else:
                          nc.scalar.activation(a1[:], p1[:], AF.Relu,
                                               bias=bn1[:, m : m + 1])
                      n1.append(a1)
                  p2 = ps1.tile([128, NC], F32, tag="ph", name=f"cp2_{c}")
                  nc.tensor.matmul(p2[:], wn2a, n1[0][:], start=True, stop=False)
                  nc.tensor.matmul(p2[:], wn2b, n1[1][:], start=False, stop=True)
                  n2 = fp_.tile([128, NC], F16, tag="n2")
                  if zb:
                      nc.scalar.activation(n2[:], p2[:], AF.Relu)
                  else:
                      nc.scalar.activation(n2[:], p2[:], AF.Relu, bias=bn2[:])
                  po = fp_.tile([128, 4 * MN], F32, tag="po")
                  for q in range(NC // 128):
                      tok0 = c0 + q * 128
                      pl = ps1.tile([128, MN], F32, tag="pi", name=f"cpl_{c}_{q}")
                      if zb:
                          nc.tensor.matmul(pl[:], n2[:, q * 128 : (q + 1) * 128], wn3)
                      else:
                          nc.tensor.matmul(
                              pl[:], n2[:, q * 128 : (q + 1) * 128], wn3,
                              start=True, stop=False,
                          )
                          nc.tensor.matmul(pl[:], ones128[:], bn3[:],
                                           start=False, stop=True)
                      ex = fp_.tile([128, MN], F32, tag="ex")
                      sm = fp_.tile([128, 1], F32, tag="sm")
                      nc.scalar.activation(ex[:], pl[:], AF.Exp, accum_out=sm[:])
                      rec = fp_.tile([128, 1], F32, tag="rec")
                      nc.vector.reciprocal(rec[:], sm[:])
                      msk = fp_.tile([128, MN], F32, tag="msk")
                      nc.vector.tensor_scalar(
                          msk[:], iota24, rnq[tok0 // 128], None, OP.is_lt
                      )
                      nc.vector.scalar_tensor_tensor(
                          po[:, q * MN : (q + 1) * MN], ex[:], rec[:], msk[:],
                          OP.mult, OP.mult
                      )
                  nc.sync.dma_start(
                      d_out.ap()[:, c * 4 * MN : (c + 1) * 4 * MN], po[:]
                  )

    nc.compile()
    return nc


def _prep_inputs(inputs, zb):
    """Host-side preprocessing -> list of per-core input dicts."""
    state = np.ascontiguousarray(inputs["state"], dtype=np.float32)
    rn = state[:, :MR]                                    # [B, 48]
    cust = state[:, MR:].reshape(B, MR, MN, FEAT)

    def f32(x):
        return np.ascontiguousarray(np.asarray(x, dtype=np.float32))

    Wih0 = f32(inputs["Wih0"]); Whh0 = f32(inputs["Whh0"])
    Wih1 = f32(inputs["Wih1"]); Whh1 = f32(inputs["Whh1"])

    sel = np.zeros((BLOC, S), np.float32)
    sel[np.arange(S) // MR, np.arange(S)] = 1.0

    w16v = {
        "wc1": np.asarray(inputs["Wc1"], np.float16),
        "wc2": np.asarray(inputs["Wc2"], np.float16),
        "ident": np.eye(128, dtype=np.float16),
        "wih0": np.tile(Wih0.astype(np.float16), (4, 1)),
        "whh0": Whh0.astype(np.float16),
        "wih1": Wih1.astype(np.float16),
        "whh1": Whh1.astype(np.float16),
        "wn1a": f32(inputs["Wn1"])[0:GH, :].astype(np.float16),
        "wn1b": (f32(inputs["Wn1"])[GH:, :] / np.float32(MR)).astype(np.float16),
        "wn2a": f32(inputs["Wn2"])[0:128, :].astype(np.float16),
        "wn2b": f32(inputs["Wn2"])[128:256, :].astype(np.float16),
        "wn3": np.asarray(inputs["Wn3"], np.float16),
        "sel": sel.astype(np.float16),
    }
    wpk16 = np.zeros((128, COLS16), np.float16)
    for name, (off, ncol, nrow) in _P16.items():
        v = w16v[name]
        assert v.shape == (nrow, ncol), (name, v.shape, (nrow, ncol))
        wpk16[:nrow, off : off + ncol] = v

    wmini = np.zeros((128, CH + CO), np.float16)
    wmini[:FEAT, 0:CH] = w16v["wc1"]
    wmini[:, CH : CH + CO] = w16v["wc2"]
    com = {"wpk16": wpk16, "wmini": wmini}
    if not zb:
        bih0 = f32(inputs["bih0"]); bhh0 = f32(inputs["bhh0"])
        bih1 = f32(inputs["bih1"]); bhh1 = f32(inputs["bhh1"])
        com.update({
            "bc1": f32(inputs["bc1"]).reshape(CH, 1),
            "bc2s": np.tile(f32(inputs["bc2"]).reshape(CO), 4).reshape(128, 1),
            "b0_r": (bih0[0:GH] + bhh0[0:GH]).reshape(GH, 1),
            "b0_z": (bih0[GH : 2 * GH] + bhh0[GH : 2 * GH]).reshape(GH, 1),
            "b0_in": bih0[2 * GH :].reshape(GH, 1),
            "b0_hn": bhh0[2 * GH :].reshape(GH, 1),
            "b1_r": (bih1[0:GH] + bhh1[0:GH]).reshape(GH, 1),
            "b1_z": (bih1[GH : 2 * GH] + bhh1[GH : 2 * GH]).reshape(GH, 1),
            "b1_in": bih1[2 * GH :].reshape(GH, 1),
            "b1_hn": bhh1[2 * GH :].reshape(GH, 1),
            "bn1c": np.ascontiguousarray(f32(inputs["bn1"]).reshape(2, 128).T),
            "bn2c": f32(inputs["bn2"]).reshape(128, 1),
            "bn3r": f32(inputs["bn3"]).reshape(1, MN),
            "ones128": np.ones((1, 128), np.float32),
        })

    in_maps = []
    for core in range(NCORES):
        b0, b1 = core * BLOC, (core + 1) * BLOC
        # cust_fm[f, n*S + (b*MR+r)] = cust[b, r, n, f]
        cfm = cust[b0:b1].transpose(3, 2, 0, 1).reshape(FEAT, MN * S)
        m = dict(com)
        m["cust_fm"] = np.ascontiguousarray(cfm.astype(np.float16))
        cpk32 = np.zeros((128, COLS32), np.float32)
        cpk32[:, 0:MN] = np.arange(MN, dtype=np.float32)[None, :]
        # rn_pm[p, q] = route_num of token q*128+p
        cpk32[:, MN:] = rn[b0:b1].reshape(S).reshape(NQ, 128).T
        m["cpk32"] = cpk32
        in_maps.append(m)
    return in_maps


def _zb(inputs):
    return all(
        float(np.abs(np.asarray(inputs[k], np.float32)).max()) == 0.0
        for k in ("bc1", "bc2", "bih0", "bhh0", "bih1", "bhh1",
                  "bn1", "bn2", "bn3")
    )


def _run(inputs, **kw):
    zb = _zb(inputs)
    key = ("nc", zb)
    if key not in _cache:
        _cache[key] = _build(zb=zb)
    nc = _cache[key]
    _cache["nc"] = nc  # for test harness introspection
    in_maps = _prep_inputs(inputs, zb)
    return run_bass_kernel_spmd(nc, in_maps, core_ids=list(range(NCORES)), **kw)


def kernel(**inputs) -> np.ndarray:
    res = _run(inputs)
    outs = []
    for r in res.results:
        pm = r["out_pm"]                                  # [128, NQ*MN]
        outs.append(pm.reshape(128, NQ, MN).transpose(1, 0, 2).reshape(S, MN))
    return np.concatenate(outs, axis=0).reshape(B, MR, MN)


# revision 47
# speedup vs baseline: 1.0049x; 1.0003x over previous
"""Trainium2 Bass kernel for nn_MLP_Route_RL_Model (route RL model).

Reference math (per batch element b of 256):
  - state = [route_nums (48) | customers (48*24*36)]
  - customer MLP (tanh-tanh, 36->128->32) on every node of every route
  - 2-layer GRU (hidden 128) over the 24 nodes of each of the 48 routes
  - route summary mean, node-selection MLP 256->256->128->24, masked softmax

Sharding: pure data parallel over batch B=256 -> 8 cores x 32.

Layout: feature-major activations ([feature, token] in SBUF) so matmuls
contract over the partition dim without transposes.

Schedule notes (the kernel is ACT elementwise-bound; engine cost is
free-dim elements only, so fewer/bigger ACT ops win):
  - r|z gate pre-activations share one 2-bank PSUM tile [128,1024]; with
    the (all-zero) biases dropped, ONE sigmoid covers both gates.
  - n-gate input finishes inside PSUM: after the DVE computes
    t_ = r*ph, an identity matmul accumulates I @ t_ onto the x-side
    matmul in the pi bank, deleting the separate s_ = pi + t_ DVE add.
  - h' = n + z*(h-n): 3 DVE ops/chunk at fp16 2x.
  - PSUM tags: przb (2 banks x2 slots) + ph (x2) + pi (x2) = 8 banks;
    the customer MLP borrows przb/pi slots at low scheduler priority and
    runs one 4-node group ahead of the GRU.
  - All fp16 weights ship in ONE packed DMA (SBUF views), fp32 consts in
    another; cust loads are per-group; outputs merge to one DMA per
    512-token chunk. DMA issue occupies the SP sequencer 565ns each, so
    instruction count matters.
  - A non-zero-bias fallback keeps split sigmoids + bias ports.
"""

import contextlib
import sys

import numpy as np

sys.path.insert(0, "/opt/trn_rl_repo")

import concourse.bass as bass  # noqa: E402
import concourse.bacc as bacc  # noqa: E402
import concourse.mybir as mybir  # noqa: E402
import concourse.tile as tile  # noqa: E402
from concourse.bass_utils import run_bass_kernel_spmd  # noqa: E402

F32 = mybir.dt.float32
F16 = mybir.dt.float16
AF = mybir.ActivationFunctionType
OP = mybir.AluOpType

# Problem shape constants
B = 256
NCORES = 8
BLOC = B // NCORES          # 32 batch rows per core
MR = 48                     # routes per batch
MN = 24                     # nodes per route
FEAT = 36
CH = 128                    # customer hidden
CO = 32                     # customer out
GH = 128                    # GRU hidden
S = BLOC * MR               # sequences per core = 1536
NC = 512                    # token chunk (PSUM bank width in fp32)
NCH = S // NC               # chunks per core = 3
NG = MN // 4                # node groups of 4 (cust_out partition stacking)
NQ = S // 128               # 128-token groups per core = 12

# fp16 weight pack layout: (col_offset, n_cols, n_rows)
_P16 = {}
_c = 0
for _name, _ncol, _nrow in [
    ("wc1", CH, FEAT), ("wc2", CO, CH), ("ident", 128, 128),
    ("wih0", 3 * GH, 128), ("whh0", 3 * GH, GH),
    ("wih1", 3 * GH, GH), ("whh1", 3 * GH, GH),
    ("wn1a", 256, GH), ("wn1b", 256, GH),
    ("wn2a", 128, 128), ("wn2b", 128, 128), ("wn3", MN, GH),
    ("sel", S, BLOC),
]:
    _P16[_name] = (_c, _ncol, _nrow)
    _c += _ncol
COLS16 = _c
# fp32 const pack layout
_P32 = {"iota24": (0, MN, 128), "rn_pm": (MN, NQ, 128)}
COLS32 = MN + NQ

_cache = {}


def _build(reps=1, zb=True):
    """Trace + schedule the per-core Tile kernel. zb: all biases are zero."""
    nc = bacc.Bacc("TRN2", target_bir_lowering=False, debug=False)

    # ---- DRAM I/O ----------------------------------------------------------
    d_cust = nc.dram_tensor("cust_fm", [FEAT, MN * S], F16, kind="ExternalInput")
    d_wmini = nc.dram_tensor("wmini", [128, CH + CO], F16, kind="ExternalInput")
    d_w16 = nc.dram_tensor("wpk16", [128, COLS16], F16, kind="ExternalInput")
    d_c32 = nc.dram_tensor("cpk32", [128, COLS32], F32, kind="ExternalInput")
    if not zb:
        d_bc1 = nc.dram_tensor("bc1", [CH, 1], F32, kind="ExternalInput")
        d_bc2 = nc.dram_tensor("bc2s", [128, 1], F32, kind="ExternalInput")
        d_gb = {}
        for layer in (0, 1):
            for g in ("r", "z", "in", "hn"):
                d_gb[(layer, g)] = nc.dram_tensor(
                    f"b{layer}_{g}", [GH, 1], F32, kind="ExternalInput"
                )
        d_bn1 = nc.dram_tensor("bn1c", [128, 2], F32, kind="ExternalInput")
        d_bn2 = nc.dram_tensor("bn2c", [128, 1], F32, kind="ExternalInput")
        d_bn3 = nc.dram_tensor("bn3r", [1, MN], F32, kind="ExternalInput")
        d_ones = nc.dram_tensor("ones128", [1, 128], F32, kind="ExternalInput")
    # partition-major output: out_pm[p, q*MN+m] = probs for token q*128+p
    d_out = nc.dram_tensor("out_pm", [128, NQ * MN], F32, kind="ExternalOutput")

    with tile.TileContext(nc) as tc:
        with (
            tc.tile_pool(name="wpool", bufs=1) as wp,
            tc.tile_pool(name="state", bufs=1) as sp,
            tc.tile_pool(name="xin", bufs=3) as xp,
            tc.tile_pool(name="h1c", bufs=8) as h1p,
            tc.tile_pool(name="wk", bufs=8) as wk,
            tc.tile_pool(name="fin", bufs=4) as fp_,
            tc.tile_pool(name="ps2", bufs=2, space="PSUM") as ps2,
            tc.tile_pool(name="ps1", bufs=2, space="PSUM") as ps1,
        ):
            def lowprio():
                # deprioritize: scheduler runs these only in recurrence gaps
                return tc.high_priority(offset=-1_000_000)

            # ---- packed weights / constants ---------------------------------
            # customer-MLP weights ship in a small DMA ahead of the big pack
            # so phase A's first matmul isn't gated on the full 1MB transfer.
            wmini = wp.tile([128, CH + CO], F16, tag="wmini")
            nc.sync.dma_start(wmini[:], d_wmini.ap())
            wpk = wp.tile([128, COLS16], F16, tag="wpk16")
            nc.sync.dma_start(wpk[:], d_w16.ap())
            cpk = wp.tile([128, COLS32], F32, tag="cpk32")
            nc.sync.dma_start(cpk[:], d_c32.ap())

            def w16(name, r0=0, rows=None, c0=0, cols=None):
                off, ncol, nrow = _P16[name]
                rows = nrow if rows is None else rows
                cols = ncol if cols is None else cols
                return wpk[r0 : r0 + rows, off + c0 : off + c0 + cols]

            wc1 = wmini[0:FEAT, 0:CH]
            wc2 = wmini[0:CH, CH : CH + CO]
            ident = w16("ident")
            # GRU weight gate slices; L1 x-side per partition-quadrant k
            whh = {
                (0, g): w16("whh0", c0=g * GH, cols=GH) for g in range(3)
            }
            whh.update({
                (1, g): w16("whh1", c0=g * GH, cols=GH) for g in range(3)
            })
            wih1g = {g: w16("wih1", c0=g * GH, cols=GH) for g in range(3)}
            wih0g = {
                (k, g): w16("wih0", r0=32 * k, rows=CO, c0=g * GH, cols=GH)
                for k in range(4) for g in range(3)
            }
            wn1a = {m: w16("wn1a", c0=128 * m, cols=128) for m in range(2)}
            wn1b = w16("wn1b")
            wn2a = w16("wn2a")
            wn2b = w16("wn2b")
            wn3 = w16("wn3")
            selc = {c: w16("sel", c0=c * NC, cols=NC) for c in range(NCH)}
            iota24 = cpk[0:128, 0:MN]
            rnq = {q: cpk[0:128, MN + q : MN + q + 1] for q in range(NQ)}

            gb = {}
            if not zb:
                def wtile(dram, shape, dtype):
                    t = wp.tile(shape, dtype, tag=dram.name)
                    nc.sync.dma_start(t[:], dram.ap())
                    return t
                bc1 = wtile(d_bc1, [CH, 1], F32)
                bc2 = wtile(d_bc2, [128, 1], F32)
                for k, d in d_gb.items():
                    gb[k] = wtile(d, [GH, 1], F32)
                bn1 = wtile(d_bn1, [128, 2], F32)
                bn2 = wtile(d_bn2, [128, 1], F32)
                bn3 = wtile(d_bn3, [1, MN], F32)
                ones128 = wtile(d_ones, [1, 128], F32)

            # persistent state: customer-MLP output, GRU hidden states
            # cust_out layout: partition = (n%4)*32 + f, free = (n//4)*S + s
            cust = sp.tile([128, NG * S], F16, tag="cust_out")
            h1 = sp.tile([GH, S], F16, tag="h1")
            h2 = sp.tile([GH, S], F16, tag="h2")

          # timing-calibration repeat loop (reps=1 in production)
          # fmt: off
            for _rep in range(reps):
              nc.vector.memset(h1[:], 0.0)
              nc.gpsimd.memset(h2[:], 0.0)

              # ---- phase A: customer MLP (gap filler) ----------------------
              # p1 pre-activations for node pairs share a przb 2-bank tile;
              # one tanh covers both (bc1 is per-partition so this also
              # holds in the non-zb fallback).
              xtiles = {}
              def emitA(g, lowp, sbs=None):
                with (lowprio() if lowp else contextlib.nullcontext()):
                  if g not in xtiles:
                      xg = xp.tile([FEAT, 4 * S], F16, tag="xc", name=f"xg{g}")
                      with (tc.high_priority() if g == 0
                            else contextlib.nullcontext()):
                          nc.sync.dma_start(
                              xg[:], d_cust.ap()[:, 4 * g * S : 4 * (g + 1) * S]
                          )
                      xtiles[g] = xg
                  xg = xtiles[g]
                  for sb in (range(NCH) if sbs is None else sbs):
                      # stage 1: h1c for the 4 nodes (2 przb borrows, one at
                      # a time); stage 2: c2 accumulation (1 pi borrow).
                      h1cbs = []
                      for kp in range(2):
                          p1b = ps2.tile([CH, 2 * NC], F32, tag="przb",
                                         name=f"p1b_{g}_{sb}_{kp}")
                          h1cb = h1p.tile([CH, 2 * NC], F16, tag="h1c")
                          for kk in range(2):
                              k = 2 * kp + kk
                              nc.tensor.matmul(
                                  p1b[:, kk * NC : (kk + 1) * NC], wc1,
                                  xg[:, k * S + sb * NC : k * S + (sb + 1) * NC],
                              )
                          if zb:
                              nc.scalar.activation(h1cb[:], p1b[:], AF.Tanh)
                          else:
                              nc.scalar.activation(h1cb[:], p1b[:], AF.Tanh, bias=bc1[:])
                          h1cbs.append(h1cb)
                      c2 = ps1.tile([128, NC], F32, tag="pi", name=f"c2_{g}_{sb}")
                      for k in range(4):
                          nc.tensor.matmul(
                              c2[32 * k : 32 * (k + 1), :], wc2,
                              h1cbs[k // 2][:, (k % 2) * NC : (k % 2 + 1) * NC],
                              tile_position=(0, 32 * k),
                          )
                      if zb:
                          nc.scalar.activation(
                              cust[:, g * S + sb * NC : g * S + (sb + 1) * NC],
                              c2[:], AF.Tanh,
                          )
                      else:
                          nc.scalar.activation(
                              cust[:, g * S + sb * NC : g * S + (sb + 1) * NC],
                              c2[:], AF.Tanh, bias=bc2[:],
                          )

              # ---- phase B: 2-layer GRU over MN steps -----------------------
              def emitB_layer(t, layer, h, kq):
                  """One GRU layer update for step t on hidden h [GH, S]."""
                  g = t // 4
                  for c in range(NCH):
                      c0, c1 = c * NC, (c + 1) * NC
                      hc = h[:, c0:c1]
                      przb = ps2.tile([GH, 2 * NC], F32, tag="przb")
                      pr = przb[:, 0:NC]
                      pz = przb[:, NC : 2 * NC]
                      ph = ps1.tile([GH, NC], F32, tag="ph")
                      pi = ps1.tile([GH, NC], F32, tag="pi")
                      if kq is not None:
                          tp = (32 * kq, 0)
                          xc = cust[32 * kq : 32 * kq + CO, g * S + c0 : g * S + c1]
                          wx = {gg: wih0g[(kq, gg)] for gg in range(3)}
                      else:
                          tp = None
                          xc = h1[:, c0:c1]
                          wx = wih1g
                      mmkw = {} if tp is None else {"tile_position": tp}
                      nc.tensor.matmul(pr, whh[(layer, 0)], hc, start=True, stop=False)
                      nc.tensor.matmul(pr, wx[0], xc, start=False, stop=True, **mmkw)
                      nc.tensor.matmul(pz, whh[(layer, 1)], hc, start=True, stop=False)
                      nc.tensor.matmul(pz, wx[1], xc, start=False, stop=True, **mmkw)
                      nc.tensor.matmul(ph[:], whh[(layer, 2)], hc)
                      rz = wk.tile([GH, 2 * NC], F16, tag="rz")
                      if zb:
                          nc.scalar.activation(rz[:], przb[:], AF.Sigmoid)
                      else:
                          nc.scalar.activation(rz[:, 0:NC], pr, AF.Sigmoid,
                                               bias=gb[(layer, "r")][:])
                          nc.scalar.activation(rz[:, NC : 2 * NC], pz, AF.Sigmoid,
                                               bias=gb[(layer, "z")][:])
                      # the t_ -> inject chain feeds the next tanh, which is
                      # the ACT critical path; lift it over queued d_/e_/h'
                      # work from earlier cells.
                      t_c = wk.tile([GH, NC], F16, tag="t_")
                      with tc.high_priority(offset=64):
                          if zb:
                              nc.vector.tensor_mul(t_c[:], ph[:], rz[:, 0:NC])
                          else:
                              nc.vector.scalar_tensor_tensor(
                                  t_c[:], ph[:], gb[(layer, "hn")][:], rz[:, 0:NC],
                                  OP.add, OP.mult,
                              )
                          # n-gate input finishes inside PSUM: pi = I@t_ + Wih_n@x
                          nc.tensor.matmul(pi[:], ident, t_c[:], start=True, stop=False)
                          nc.tensor.matmul(pi[:], wx[2], xc, start=False, stop=True, **mmkw)
                      n_c = wk.tile([GH, NC], F16, tag="n")
                      if zb:
                          nc.scalar.activation(n_c[:], pi[:], AF.Tanh)
                      else:
                          nc.scalar.activation(n_c[:], pi[:], AF.Tanh,
                                               bias=gb[(layer, "in")][:])
                      # h' = n + z*(h - n), all DVE at fp16 2x
                      d_c = wk.tile([GH, NC], F16, tag="d_")
                      nc.vector.tensor_sub(d_c[:], hc, n_c[:])
                      e_c = wk.tile([GH, NC], F16, tag="e_")
                      nc.vector.tensor_mul(e_c[:], rz[:, NC : 2 * NC], d_c[:])
                      nc.vector.tensor_add(hc, n_c[:], e_c[:])

              # customer MLP runs one 4-node group AHEAD of the GRU, spread
              # one 512-token chunk per step so every step has ACT filler
              # work and a group boundary never stalls L1.
              emitA(0, lowp=False)
              for t in range(MN):
                  g_next, sb = t // 4 + 1, t % 4
                  if g_next < NG and sb < NCH:
                      emitA(g_next, lowp=True, sbs=[sb])
                  emitB_layer(t, 0, h1, t % 4)
                  emitB_layer(t, 1, h2, None)

              # ---- phase C: route mean + node MLP + masked softmax ----------
              h2v = h2[:].rearrange("p (b r) -> p b r", r=MR)
              mean = fp_.tile([GH, BLOC], F16, tag="mean")
              with nc.allow_low_precision(reason="route-mean in fp16; rel tol 2e-2"):
                  nc.vector.tensor_reduce(mean[:], h2v, mybir.AxisListType.X, OP.add)
              pmt = ps1.tile([BLOC, 256], F32, tag="ph", name="cpmt")
              nc.tensor.matmul(pmt[:], mean[:], wn1b)
              mmt = fp_.tile([BLOC, 256], F16, tag="mmt")
              nc.vector.tensor_copy(mmt[:], pmt[:])

              for c in range(NCH):
                  c0, c1 = c * NC, (c + 1) * NC
                  p1b = ps2.tile([128, 2 * NC], F32, tag="przb", name=f"cp1_{c}")
                  for m in range(2):
                      p1 = p1b[:, m * NC : (m + 1) * NC]
                      nc.tensor.matmul(
                          p1, wn1a[m], h2[:, c0:c1], start=True, stop=False,
                      )
                      nc.tensor.matmul(
                          p1, mmt[:, 128 * m : 128 * (m + 1)], selc[c],
                          start=False, stop=True,
                      )
                  a1 = fp_.tile([128, 2 * NC], F16, tag="n1")
                  if zb:
                      nc.scalar.activation(a1[:], p1b[:], AF.Relu)
                  else:
                      nc.scalar.activation(a1[:, 0:NC], p1b[:, 0:NC], AF.Relu,
                                           bias=bn1[:, 0:1])
                      nc.scalar.activation(a1[:, NC : 2 * NC], p1b[:, NC : 2 * NC],
                                           AF.Relu, bias=bn1[:, 1:2])
                  p2 = ps1.tile([128, NC], F32, tag="ph", name=f"cp2_{c}")
                  nc.tensor.matmul(p2[:], wn2a, a1[:, 0:NC], start=True, stop=False)
                  nc.tensor.matmul(p2[:], wn2b, a1[:, NC : 2 * NC], start=False, stop=True)
                  n2 = fp_.tile([128, NC], F16, tag="n2")
                  if zb:
                      nc.scalar.activation(n2[:], p2[:], AF.Relu)
                  else:
                      nc.scalar.activation(n2[:], p2[:], AF.Relu, bias=bn2[:])
                  po = fp_.tile([128, 4 * MN], F32, tag="po")
                  for q in range(NC // 128):
                      tok0 = c0 + q * 128
                      pl = ps1.tile([128, MN], F32, tag="pi", name=f"cpl_{c}_{q}")
                      if zb:
                          nc.tensor.matmul(pl[:], n2[:, q * 128 : (q + 1) * 128], wn3)
                      else:
                          nc.tensor.matmul(
                              pl[:], n2[:, q * 128 : (q + 1) * 128], wn3,
                              start=True, stop=False,
                          )
                          nc.tensor.matmul(pl[:], ones128[:], bn3[:],
                                           start=False, stop=True)
                      ex = fp_.tile([128, MN], F32, tag="ex")
                      sm = fp_.tile([128, 1], F32, tag="sm")
                      nc.scalar.activation(ex[:], pl[:], AF.Exp, accum_out=sm[:])
                      rec = fp_.tile([128, 1], F32, tag="rec")
                      nc.vector.reciprocal(rec[:], sm[:])
                      msk = fp_.tile([128, MN], F32, tag="msk")
                      nc.vector.tensor_scalar(
                          msk[:], iota24, rnq[tok0 // 128], None, OP.is_lt
                      )
                      nc.vector.scalar_tensor_tensor(
                          po[:, q * MN : (q + 1) * MN], ex[:], rec[:], msk[:],
                          OP.mult, OP.mult
                      )
                  nc.sync.dma_start(
                      d_out.ap()[:, c * 4 * MN : (c + 1) * 4 * MN], po[:]
                  )

    nc.compile()
    return nc


def _prep_inputs(inputs, zb):
    """Host-side preprocessing -> list of per-core input dicts."""
    state = np.ascontiguousarray(inputs["state"], dtype=np.float32)
    rn = state[:, :MR]                                    # [B, 48]
    cust = state[:, MR:].reshape(B, MR, MN, FEAT)

    def f32(x):
        return np.ascontiguousarray(np.asarray(x, dtype=np.float32))

    Wih0 = f32(inputs["Wih0"]); Whh0 = f32(inputs["Whh0"])
    Wih1 = f32(inputs["Wih1"]); Whh1 = f32(inputs["Whh1"])

    sel = np.zeros((BLOC, S), np.float32)
    sel[np.arange(S) // MR, np.arange(S)] = 1.0

    w16v = {
        "wc1": np.asarray(inputs["Wc1"], np.float16),
        "wc2": np.asarray(inputs["Wc2"], np.float16),
        "ident": np.eye(128, dtype=np.float16),
        "wih0": np.tile(Wih0.astype(np.float16), (4, 1)),
        "whh0": Whh0.astype(np.float16),
        "wih1": Wih1.astype(np.float16),
        "whh1": Whh1.astype(np.float16),
        "wn1a": f32(inputs["Wn1"])[0:GH, :].astype(np.float16),
        "wn1b": (f32(inputs["Wn1"])[GH:, :] / np.float32(MR)).astype(np.float16),
        "wn2a": f32(inputs["Wn2"])[0:128, :].astype(np.float16),
        "wn2b": f32(inputs["Wn2"])[128:256, :].astype(np.float16),
        "wn3": np.asarray(inputs["Wn3"], np.float16),
        "sel": sel.astype(np.float16),
    }
    wpk16 = np.zeros((128, COLS16), np.float16)
    for name, (off, ncol, nrow) in _P16.items():
        v = w16v[name]
        assert v.shape == (nrow, ncol), (name, v.shape, (nrow, ncol))
        wpk16[:nrow, off : off + ncol] = v

    wmini = np.zeros((128, CH + CO), np.float16)
    wmini[:FEAT, 0:CH] = w16v["wc1"]
    wmini[:, CH : CH + CO] = w16v["wc2"]
    com = {"wpk16": wpk16, "wmini": wmini}
    if not zb:
        bih0 = f32(inputs["bih0"]); bhh0 = f32(inputs["bhh0"])
        bih1 = f32(inputs["bih1"]); bhh1 = f32(inputs["bhh1"])
        com.update({
            "bc1": f32(inputs["bc1"]).reshape(CH, 1),
            "bc2s": np.tile(f32(inputs["bc2"]).reshape(CO), 4).reshape(128, 1),
            "b0_r": (bih0[0:GH] + bhh0[0:GH]).reshape(GH, 1),
            "b0_z": (bih0[GH : 2 * GH] + bhh0[GH : 2 * GH]).reshape(GH, 1),
            "b0_in": bih0[2 * GH :].reshape(GH, 1),
            "b0_hn": bhh0[2 * GH :].reshape(GH, 1),
            "b1_r": (bih1[0:GH] + bhh1[0:GH]).reshape(GH, 1),
            "b1_z": (bih1[GH : 2 * GH] + bhh1[GH : 2 * GH]).reshape(GH, 1),
            "b1_in": bih1[2 * GH :].reshape(GH, 1),
            "b1_hn": bhh1[2 * GH :].reshape(GH, 1),
            "bn1c": np.ascontiguousarray(f32(inputs["bn1"]).reshape(2, 128).T),
            "bn2c": f32(inputs["bn2"]).reshape(128, 1),
            "bn3r": f32(inputs["bn3"]).reshape(1, MN),
            "ones128": np.ones((1, 128), np.float32),
        })

    in_maps = []
    for core in range(NCORES):
        b0, b1 = core * BLOC, (core + 1) * BLOC
        # cust_fm[f, n*S + (b*MR+r)] = cust[b, r, n, f]
        cfm = cust[b0:b1].transpose(3, 2, 0, 1).reshape(FEAT, MN * S)
        m = dict(com)
        m["cust_fm"] = np.ascontiguousarray(cfm.astype(np.float16))
        cpk32 = np.zeros((128, COLS32), np.float32)
        cpk32[:, 0:MN] = np.arange(MN, dtype=np.float32)[None, :]
        # rn_pm[p, q] = route_num of token q*128+p
        cpk32[:, MN:] = rn[b0:b1].reshape(S).reshape(NQ, 128).T
        m["cpk32"] = cpk32
        in_maps.append(m)
    return in_maps


def _zb(inputs):
    return all(
        float(np.abs(np.asarray(inputs[k], np.float32)).max()) == 0.0
        for k in ("bc1", "bc2", "bih0", "bhh0", "bih1", "bhh1",
                  "bn1", "bn2", "bn3")
    )


def _run(inputs, **kw):
    zb = _zb(inputs)
    key = ("nc", zb)
    if key not in _cache:
        _cache[key] = _build(zb=zb)
    nc = _cache[key]
    _cache["nc"] = nc  # for test harness introspection
    in_maps = _prep_inputs(inputs, zb)
    return run_bass_kernel_spmd(nc, in_maps, core_ids=list(range(NCORES)), **kw)


def kernel(**inputs) -> np.ndarray:
    res = _run(inputs)
    outs = []
    for r in res.results:
        pm = r["out_pm"]                                  # [128, NQ*MN]
        outs.append(pm.reshape(128, NQ, MN).transpose(1, 0, 2).reshape(S, MN))
    return np.concatenate(outs, axis=0).reshape(B, MR, MN)
